# revision 6
# baseline (speedup 1.0000x reference)
"""Bass/Trainium2 kernel for nn_KVPosAttentionMapping.

Reference computation (N == M so tmp = keys):
    scores   = einsum('bhnd,bhmd->bhnm', keys, keys) / sqrt(H)
    pos_term = einsum('onmp,p->onm', pos_enc, w_pos)
    aw       = scores * sum(w_pos) + pos_term + b_pos[0]    -> [B*H, N, M]
    returns (aw, q, k, v) with q/k/v plain reshapes of the inputs.

Distribution: shard the query-row axis N across the 8 cores (128 rows each,
for all 64 batch*head pairs). This minimizes traffic versus bh-sharding,
which would replicate the 192MB pos_enc read on every core.

Per-core device work:
    pos_term[n,m] = reduce_p(posw[n,m,p])            (w_pos/b folded on host)
    for bh: scores = tmpT^T @ keysT  (TensorE), aw = scores + pos_term (DVE)

Host prep: pre-transpose keys to [bh, d, m] (lhsT/rhs layouts), fold the
scalar scale sum(w)/sqrt(H) into the tmp operand, fold w_pos and b_pos into
pos_enc, and cast everything to fp16 (inputs are ~N(0,1); fp16 keeps the
final L2 relative error ~1e-3, far inside tolerance, and halves DMA bytes).
"""

import numpy as np

B, H, N, M, DQ, DV, P = 4, 16, 1024, 1024, 64, 64, 48
BH = B * H
NCORES = 8
NS = N // NCORES  # 128 query rows per core

OUT_GROUP = 8  # bh per output DMA batch
POS_MB = 256   # m-block for the pos reduction

_CACHE = {}


def _build_bass():
    from contextlib import ExitStack

    import concourse.mybir as mybir
    import concourse.tile as tile
    from concourse import bacc

    fp16 = mybir.dt.float16
    f32 = mybir.dt.float32

    # Bacc (not plain Bass): its compile() pass splits multi-sem waits into
    # event-semaphore instructions — raw DMA instructions allow only 1 wait.
    nc = bacc.Bacc()
    keysT = nc.declare_dram_parameter("keysT", [BH, DQ, M], fp16, isOutput=False)
    tmpT = nc.declare_dram_parameter("tmpT", [BH, DQ, NS], fp16, isOutput=False)
    posw = nc.declare_dram_parameter("posw", [NS, M, P], fp16, isOutput=False)
    # out layout [n, bh, m] so an OUT_GROUP of bh is contiguous per partition
    awo = nc.declare_dram_parameter("awo", [NS, BH, M], fp16, isOutput=True)

    with ExitStack() as ctx:
        tc = ctx.enter_context(tile.TileContext(nc))
        pos_pool = ctx.enter_context(tc.tile_pool(name="pos", bufs=2))
        pterm_pool = ctx.enter_context(tc.tile_pool(name="pterm", bufs=1))
        k_pool = ctx.enter_context(tc.tile_pool(name="keys", bufs=3))
        t_pool = ctx.enter_context(tc.tile_pool(name="tmp", bufs=1))
        psum_pool = ctx.enter_context(tc.tile_pool(name="psum", bufs=4, space="PSUM"))
        out_pool = ctx.enter_context(tc.tile_pool(name="out", bufs=3))

        # ---- pos_term [128, 1024] f32, resident ----
        pos_term = pterm_pool.tile([NS, M], f32)
        for mb in range(M // POS_MB):
            pt = pos_pool.tile([NS, POS_MB, P], fp16)
            nc.gpsimd.dma_start(out=pt[:], in_=posw[:, mb * POS_MB:(mb + 1) * POS_MB, :])
            nc.vector.tensor_reduce(
                out=pos_term[:, mb * POS_MB:(mb + 1) * POS_MB],
                in_=pt[:],
                axis=mybir.AxisListType.X,
                op=mybir.AluOpType.add,
            )

        # ---- tmpT resident: partition = (bh%2)*64 + d, free = (bh//2, n) ----
        tmpT_sb = t_pool.tile([128, BH // 2, NS], fp16)
        nc.gpsimd.dma_start(
            out=tmpT_sb[:],
            in_=tmpT.rearrange("(q t) d n -> (t d) q n", t=2),
        )

        # ---- main loop over bh ----
        for g in range(BH // OUT_GROUP):
            ot = out_pool.tile([NS, OUT_GROUP * M], fp16)
            for bi in range(OUT_GROUP):
                bh = g * OUT_GROUP + bi
                q, t = bh // 2, bh % 2
                if t == 0:
                    kt = k_pool.tile([128, M], fp16, tag="kt")
                    nc.gpsimd.dma_start(
                        out=kt[:],
                        in_=keysT[2 * q:2 * q + 2].rearrange("t d m -> (t d) m"),
                    )
                lhsT = tmpT_sb[t * DQ:(t + 1) * DQ, q, :]
                for j in range(M // 512):
                    ps = psum_pool.tile([NS, 512], f32)
                    nc.tensor.matmul(
                        out=ps[:],
                        lhsT=lhsT,
                        rhs=kt[t * DQ:(t + 1) * DQ, j * 512:(j + 1) * 512],
                        start=True,
                        stop=True,
                    )
                    nc.vector.tensor_add(
                        out=ot[:, bi * M + j * 512: bi * M + (j + 1) * 512],
                        in0=ps[:],
                        in1=pos_term[:, j * 512:(j + 1) * 512],
                    )
            nc.gpsimd.dma_start(
                out=awo[:, g * OUT_GROUP:(g + 1) * OUT_GROUP, :],
                in_=ot.rearrange("n (b m) -> n b m", b=OUT_GROUP),
            )
    nc.finalize()
    return nc


def _prep_inputs(keys, pos_enc, w_pos, b_pos):
    """Host-side marshalling into the per-core fp16 shard arrays."""
    scale = float(np.sum(w_pos.astype(np.float64))) / float(np.sqrt(H))

    keys_bh = np.ascontiguousarray(
        keys.reshape(BH, M, DQ).transpose(0, 2, 1)
    )  # [BH, DQ, M] f32
    keysT16 = keys_bh.astype(np.float16)

    posw = pos_enc[0].astype(np.float32) * w_pos.astype(np.float32)[None, None, :]
    posw[..., 0] += float(b_pos[0])

    in_maps = []
    for c in range(NCORES):
        sl = slice(c * NS, (c + 1) * NS)
        tmpT_c = (keys_bh[:, :, sl] * scale).astype(np.float16)
        posw_c = posw[sl].astype(np.float16)
        in_maps.append({
            "keysT": keysT16,
            "tmpT": np.ascontiguousarray(tmpT_c),
            "posw": np.ascontiguousarray(posw_c),
        })
    return in_maps


def run(queries, keys, values, pos_enc, w_pos, b_pos, trace=False, trace_kwargs=None):
    from concourse.bass_utils import run_bass_kernel_spmd

    if "nc" not in _CACHE:
        _CACHE["nc"] = _build_bass()
    nc = _CACHE["nc"]

    in_maps = _prep_inputs(keys, pos_enc, w_pos, b_pos)
    kw = {}
    if trace:
        kw["trace"] = True
        if trace_kwargs:
            kw.update(trace_kwargs)
    res = run_bass_kernel_spmd(nc, in_maps, list(range(NCORES)), **kw)

    # [NCORES, NS, BH, M] -> aw [BH, N, M] f32
    awo = np.stack([r["awo"] for r in res.results], axis=0)
    aw = np.ascontiguousarray(
        awo.reshape(N, BH, M).transpose(1, 0, 2)
    ).astype(np.float32)

    q = queries.reshape(BH, N, DQ).astype(np.float32, copy=False)
    k = keys.reshape(BH, M, DQ).astype(np.float32, copy=False)
    v = values.reshape(BH, M, DV).astype(np.float32, copy=False)
    return (aw, q, k, v), res


def kernel(queries, keys, values, pos_enc, w_pos, b_pos, **_unused):
    queries = np.asarray(queries, dtype=np.float32)
    keys = np.asarray(keys, dtype=np.float32)
    values = np.asarray(values, dtype=np.float32)
    pos_enc = np.asarray(pos_enc, dtype=np.float32)
    w_pos = np.asarray(w_pos, dtype=np.float32)
    b_pos = np.asarray(b_pos, dtype=np.float32)
    outs, _ = run(queries, keys, values, pos_enc, w_pos, b_pos, trace=False)
    return outs


# revision 9
# speedup vs baseline: 1.1452x; 1.1452x over previous
"""Bass/Trainium2 kernel for nn_KVPosAttentionMapping.

Reference computation (N == M so tmp = keys):
    scores   = einsum('bhnd,bhmd->bhnm', keys, keys) / sqrt(H)
    pos_term = einsum('onmp,p->onm', pos_enc, w_pos)
    aw       = scores * sum(w_pos) + pos_term + b_pos[0]    -> [B*H, N, M]
    returns (aw, q, k, v) with q/k/v plain reshapes of the inputs.

Distribution: shard the query-row axis N across the 8 cores (128 rows each,
for all 64 batch*head pairs). This minimizes traffic versus bh-sharding,
which would replicate the 192MB pos_enc read on every core.

Per-core device work:
    pos_term[n,m] = reduce_p(posw[n,m,p])            (w_pos/b folded on host)
    for bh: scores = tmpT^T @ keysT  (TensorE), aw = scores + pos_term (DVE)

Host prep: pre-transpose keys to [bh, d, m] (lhsT/rhs layouts), fold the
scalar scale sum(w)/sqrt(H) into the tmp operand, fold w_pos and b_pos into
pos_enc, and cast everything to fp16 (inputs are ~N(0,1); fp16 keeps the
final L2 relative error ~1e-3, far inside tolerance, and halves DMA bytes).
"""

import numpy as np

B, H, N, M, DQ, DV, P = 4, 16, 1024, 1024, 64, 64, 48
BH = B * H
NCORES = 8
NS = N // NCORES  # 128 query rows per core

OUT_GROUP = 8  # bh per output DMA batch
POS_PG = 8     # p-slices per pos load tile

_CACHE = {}


def _build_bass():
    from contextlib import ExitStack

    import concourse.mybir as mybir
    import concourse.tile as tile
    from concourse import bacc

    fp16 = mybir.dt.float16
    f32 = mybir.dt.float32

    # Bacc (not plain Bass): its compile() pass splits multi-sem waits into
    # event-semaphore instructions — raw DMA instructions allow only 1 wait.
    nc = bacc.Bacc()
    keysT = nc.declare_dram_parameter("keysT", [BH, DQ, M], fp16, isOutput=False)
    tmpT = nc.declare_dram_parameter("tmpT", [BH, DQ, NS], fp16, isOutput=False)
    # p-major so pos_term accumulates via contiguous fp16 SBUF adds (fast DVE
    # mode) instead of a segmented reduce (which ran at 1 elem/lane/cycle)
    posw = nc.declare_dram_parameter("posw", [P, NS, M], fp16, isOutput=False)
    # out layout [n, bh, m] so an OUT_GROUP of bh is contiguous per partition
    awo = nc.declare_dram_parameter("awo", [NS, BH, M], fp16, isOutput=True)

    with ExitStack() as ctx:
        tc = ctx.enter_context(tile.TileContext(nc))
        pos_pool = ctx.enter_context(tc.tile_pool(name="pos", bufs=2))
        pterm_pool = ctx.enter_context(tc.tile_pool(name="pterm", bufs=1))
        k_pool = ctx.enter_context(tc.tile_pool(name="keys", bufs=3))
        t_pool = ctx.enter_context(tc.tile_pool(name="tmp", bufs=1))
        psum_pool = ctx.enter_context(tc.tile_pool(name="psum", bufs=4, space="PSUM"))
        out_pool = ctx.enter_context(tc.tile_pool(name="out", bufs=3))

        # ---- pos_term [128, 1024] fp16, resident: chained adds over p ----
        pos_term = pterm_pool.tile([NS, M], fp16)
        ptiles = []
        for pg in range(P // POS_PG):
            pt = pos_pool.tile([NS, POS_PG, M], fp16, tag="pt")
            nc.sync.dma_start(
                out=pt[:], in_=posw[pg * POS_PG:(pg + 1) * POS_PG].rearrange("p n m -> n p m")
            )
            ptiles.append(pt)
            if pg == 0:
                nc.vector.tensor_add(out=pos_term[:], in0=pt[:, 0, :], in1=pt[:, 1, :])
                rest = range(2, POS_PG)
            else:
                rest = range(POS_PG)
            for i in rest:
                nc.vector.tensor_add(out=pos_term[:], in0=pos_term[:], in1=pt[:, i, :])

        # ---- tmpT resident: partition = (bh%2)*64 + d, free = (bh//2, n) ----
        tmpT_sb = t_pool.tile([128, BH // 2, NS], fp16)
        nc.sync.dma_start(
            out=tmpT_sb[:],
            in_=tmpT.rearrange("(q t) d n -> (t d) q n", t=2),
        )

        # ---- main loop over bh ----
        for g in range(BH // OUT_GROUP):
            ot = out_pool.tile([NS, OUT_GROUP * M], fp16)
            for bi in range(OUT_GROUP):
                bh = g * OUT_GROUP + bi
                q, t = bh // 2, bh % 2
                if t == 0:
                    kt = k_pool.tile([128, M], fp16, tag="kt")
                    nc.sync.dma_start(
                        out=kt[:],
                        in_=keysT[2 * q:2 * q + 2].rearrange("t d m -> (t d) m"),
                    )
                lhsT = tmpT_sb[t * DQ:(t + 1) * DQ, q, :]
                ps = psum_pool.tile([NS, M], f32)
                for j in range(M // 512):
                    nc.tensor.matmul(
                        out=ps[:, j * 512:(j + 1) * 512],
                        lhsT=lhsT,
                        rhs=kt[t * DQ:(t + 1) * DQ, j * 512:(j + 1) * 512],
                        start=True,
                        stop=True,
                    )
                nc.vector.tensor_add(
                    out=ot[:, bi * M:(bi + 1) * M],
                    in0=ps[:],
                    in1=pos_term[:],
                )
            nc.sync.dma_start(
                out=awo[:, g * OUT_GROUP:(g + 1) * OUT_GROUP, :],
                in_=ot.rearrange("n (b m) -> n b m", b=OUT_GROUP),
            )
    nc.finalize()
    return nc


def _prep_inputs(keys, pos_enc, w_pos, b_pos):
    """Host-side marshalling into the per-core fp16 shard arrays."""
    scale = float(np.sum(w_pos.astype(np.float64))) / float(np.sqrt(H))

    keys_bh = np.ascontiguousarray(
        keys.reshape(BH, M, DQ).transpose(0, 2, 1)
    )  # [BH, DQ, M] f32
    keysT16 = keys_bh.astype(np.float16)

    posw = pos_enc[0].astype(np.float32) * w_pos.astype(np.float32)[None, None, :]
    posw[..., 0] += float(b_pos[0])
    # device wants p-major [P, N, M]
    posw_t = np.ascontiguousarray(posw.transpose(2, 0, 1).astype(np.float16))

    in_maps = []
    for c in range(NCORES):
        sl = slice(c * NS, (c + 1) * NS)
        tmpT_c = (keys_bh[:, :, sl] * scale).astype(np.float16)
        in_maps.append({
            "keysT": keysT16,
            "tmpT": np.ascontiguousarray(tmpT_c),
            "posw": np.ascontiguousarray(posw_t[:, sl, :]),
        })
    return in_maps


def run(queries, keys, values, pos_enc, w_pos, b_pos, trace=False, trace_kwargs=None):
    from concourse.bass_utils import run_bass_kernel_spmd

    if "nc" not in _CACHE:
        _CACHE["nc"] = _build_bass()
    nc = _CACHE["nc"]

    in_maps = _prep_inputs(keys, pos_enc, w_pos, b_pos)
    kw = {}
    if trace:
        kw["trace"] = True
        if trace_kwargs:
            kw.update(trace_kwargs)
    res = run_bass_kernel_spmd(nc, in_maps, list(range(NCORES)), **kw)

    # [NCORES, NS, BH, M] -> aw [BH, N, M] f32
    awo = np.stack([r["awo"] for r in res.results], axis=0)
    aw = np.ascontiguousarray(
        awo.reshape(N, BH, M).transpose(1, 0, 2)
    ).astype(np.float32)

    q = queries.reshape(BH, N, DQ).astype(np.float32, copy=False)
    k = keys.reshape(BH, M, DQ).astype(np.float32, copy=False)
    v = values.reshape(BH, M, DV).astype(np.float32, copy=False)
    return (aw, q, k, v), res


def kernel(queries, keys, values, pos_enc, w_pos, b_pos, **_unused):
    queries = np.asarray(queries, dtype=np.float32)
    keys = np.asarray(keys, dtype=np.float32)
    values = np.asarray(values, dtype=np.float32)
    pos_enc = np.asarray(pos_enc, dtype=np.float32)
    w_pos = np.asarray(w_pos, dtype=np.float32)
    b_pos = np.asarray(b_pos, dtype=np.float32)
    outs, _ = run(queries, keys, values, pos_enc, w_pos, b_pos, trace=False)
    return outs
